# revision 1
# baseline (speedup 1.0000x reference)
"""Trainium2 Bass kernel for nn_BiAlignLayer.

Reference computation:
    weight   = einsum('bld,bmd->blm', i, j)
    weight_i = softmax(weight, axis=-1)   # rows sum to 1 over m
    weight_j = softmax(weight, axis=1)    # cols sum to 1 over l
    weighted_i = einsum('blm,bld->bmd', weight_i, i)
    weighted_j = einsum('blm,bmd->bld', weight_j, j)
    oi = relu(mean_l(i - weighted_j) @ W + b)
    oj = relu(mean_m(j - weighted_i) @ W + b)
    out = 0.5 * (oi + oj)

Because mean_m(weighted_i) = mean_l(i) (softmax over m sums to 1) and
mean_l(weighted_j) = mean_m(j) (softmax over l sums to 1), the whole
attention block drops out of the final means:
    u   = mean_l(i) - mean_l(j)                       # [B, D]
    out = 0.5 * (relu(u @ W + b) + relu(-(u @ W) + b))
The kernel computes exactly that, in exact fp32, and is bound by the HBM
read of i and j (16.8 MB per core at ~358 GB/s ~= 47 us):

  * Reduction over L split across engines so neither exceeds the DMA
    floor: i tiles reduce on the tensor engine (one matmul per [128,512]
    tile against a signed one-hot selector column, accumulating all 4
    batch rows in a single PSUM bank), j tiles chain-sum on the
    otherwise-idle vector engine and enter PSUM via one matmul per batch.
    Selector values are +-1/(2L) (exact powers of two), folding the mean
    and the final 0.5 into the accumulation for free.
  * W/b DMAs are queued after the data stream (they are only consumed by
    the dense tail, and this lets the last data tile land ~3 us earlier).
  * The dense layer runs in transposed [NN, B] layout; the bias enters
    PSUM as a rank-1 (K=1) matmul with a 0.5-valued rhs, and
    0.5*relu(x) == relu(0.5*x) makes the epilogue two vector-engine
    relu-max ops plus one add. A single DMA stores the [512, 4] result.

Sharding: data-parallel over batch, 4 batch elements per core x 8 cores.
"""

import sys

import numpy as np

if "/opt/trn_rl_repo" not in sys.path:
    sys.path.insert(0, "/opt/trn_rl_repo")

import concourse.mybir as mybir
import concourse.tile as tile
from concourse import bacc
from concourse.bass import ds
from concourse.bass_utils import run_bass_kernel_spmd
from concourse.masks import make_identity

B = 32            # total batch
NCORES = 8
NB = B // NCORES  # batches per core
L = 1024
D = 512
NN = 512          # output feature dim (2 * nn_dim)
P = 128
LCH = L // P      # 128-row chunks per batch element
DCH = D // P
NCH = NN // P
F32 = mybir.dt.float32

_CACHE = {}


def _build_bass(reps=1):
    """Build the per-core Bass program. reps>1 repeats the body (for the
    wall-clock marginal benchmark); outputs are simply overwritten."""
    nc = bacc.Bacc("TRN2", debug=False)

    i_dram = nc.declare_dram_parameter("i", [NB * L, D], F32, isOutput=False)
    j_dram = nc.declare_dram_parameter("j", [NB * L, D], F32, isOutput=False)
    w_dram = nc.declare_dram_parameter("w", [D, NN], F32, isOutput=False)
    b_dram = nc.declare_dram_parameter("b", [1, NN], F32, isOutput=False)
    o_dram = nc.declare_dram_parameter("out", [NN, NB], F32, isOutput=True)

    # out[cn*P + p, b] <- o_sb[p, cn*NB + b]
    o_view = o_dram.ap().rearrange("(c p) b -> p c b", p=P)

    with tile.TileContext(nc) as tc:
        with (
            tc.tile_pool(name="consts", bufs=1) as consts,
            tc.tile_pool(name="data", bufs=12) as data,
            tc.tile_pool(name="jacc", bufs=2) as jpool,
            tc.tile_pool(name="small", bufs=1) as small,
            tc.tile_pool(name="psum", bufs=1, space="PSUM") as psum,
        ):
            # Signed one-hot selectors, pre-scaled by 1/(2L) (an exact power
            # of two): sel[:, NB*(2b+0) + b] = +1/(2L) for i tiles,
            # sel[:, NB*(2b+1) + b] = -1/(2L) for the j accumulators. A
            # matmul with a selector block as stationary adds the column
            # sums of its rhs, scaled, into PSUM row b; +-1/2L weights are
            # exact under the fp32 matmul's internal decomposition.
            s = 1.0 / (2.0 * L)
            sel = consts.tile([P, NB * (2 * NB)], F32)
            nc.vector.memset(sel[:], 0.0)
            for b in range(NB):
                nc.vector.memset(sel[:, ds(NB * (2 * b) + b, 1)], s)
                nc.vector.memset(sel[:, ds(NB * (2 * b + 1) + b, 1)], -s)

            ident = consts.tile([NB, NB], F32)
            make_identity(nc, ident[:])
            halfones = consts.tile([1, NB], F32)
            nc.vector.memset(halfones[:], 0.5)

            w_sb = consts.tile([P, DCH * NN], F32)
            b_sb = consts.tile([1, NN], F32)

            for rep in range(reps):
                _emit_body(
                    nc, data, jpool, small, psum,
                    i_dram.ap(), j_dram.ap(), w_dram.ap(), b_dram.ap(),
                    o_view, sel, ident, halfones, w_sb, b_sb,
                    load_wb=(rep == 0),
                )

    nc.compile()
    return nc


def _emit_body(nc, data, jpool, small, psum, i_ap, j_ap, w_ap, b_ap,
               o_view, sel, ident, halfones, w_sb, b_sb, load_wb=True):
    # --- phase 1: u_psum[b, :] = (sum_l i[b] - sum_l j[b]) / 2L ------------
    # The fp32 PE matmul costs 4 cycles/row and the DMA stream is the real
    # floor, so the reduction is split: i tiles go straight to the PE (two
    # selector matmuls per double-row tile), j tiles are chain-summed on
    # the otherwise-idle DVE and enter PSUM via two selector matmuls per
    # batch. Exact fp32.
    #
    # Tiles pack TWO consecutive DRAM rows per partition line ([128, 2*D]),
    # making each DMA descriptor 4 KB contiguous -- the size HBM/SBUF need
    # to saturate bus width -- and the i/j streams ride separate HWDGE
    # queues (SP and ACT) so descriptor generation fans out to more DMA
    # engines.
    RPT = 2 * P          # DRAM rows per tile
    TCH = L // RPT       # tiles per batch element
    u_psum = psum.tile([NB, D], F32)
    # Per batch: i tiles lc 0..1 fold into a DVE chain (like all of j),
    # lc 2..3 go straight to the PE -- balances PE (fp32 matmul, 4 cyc/row)
    # against the DVE so neither exceeds the DMA stream.
    n_mm = NB * (2 * (TCH - 2) + 2 + 2)
    k = 0
    for b in range(NB):
        jacc = jpool.tile([P, 2 * D], F32, tag="jacc")
        iacc = jpool.tile([P, 2 * D], F32, tag="iacc")
        tj0 = None
        ti0 = None
        for lc in range(TCH):
            ti = data.tile([P, 2 * D], F32, tag="ti")
            nc.sync.dma_start(
                out=ti[:].rearrange("p (t n) -> p t n", t=2),
                in_=i_ap[ds(b * L + lc * RPT, RPT), :].rearrange(
                    "(p t) n -> p t n", t=2
                ),
            )
            if lc == 0:
                ti0 = ti
            elif lc == 1:
                nc.vector.tensor_add(iacc[:], ti0[:], ti[:])
                for t in range(2):
                    nc.tensor.matmul(
                        u_psum[:],
                        sel[:, ds(NB * (2 * b), NB)],
                        iacc[:, ds(t * D, D)],
                        start=(k == 0),
                        stop=False,
                    )
                    k += 1
            else:
                for t in range(2):
                    nc.tensor.matmul(
                        u_psum[:],
                        sel[:, ds(NB * (2 * b), NB)],
                        ti[:, ds(t * D, D)],
                        start=(k == 0),
                        stop=False,
                    )
                    k += 1
            tj = data.tile([P, 2 * D], F32, tag="tj")
            nc.scalar.dma_start(
                out=tj[:].rearrange("p (t n) -> p t n", t=2),
                in_=j_ap[ds(b * L + lc * RPT, RPT), :].rearrange(
                    "(p t) n -> p t n", t=2
                ),
            )
            if lc == 0:
                tj0 = tj
            elif lc == 1:
                nc.vector.tensor_add(jacc[:], tj0[:], tj[:])
            else:
                nc.vector.tensor_add(jacc[:], jacc[:], tj[:])
        for t in range(2):
            nc.tensor.matmul(
                u_psum[:],
                sel[:, ds(NB * (2 * b + 1), NB)],
                jacc[:, ds(t * D, D)],
                start=False,
                stop=(k == n_mm - 1),
            )
            k += 1

    # W and b are only consumed by the dense tail, so their DMAs are queued
    # AFTER the data stream: the last data tile (which gates the tail's u
    # chain) lands ~3us earlier, and W streams in while the u copy /
    # transpose work below runs.
    if load_wb:
        for c in range(DCH):
            eng = nc.sync if c % 2 == 0 else nc.scalar
            eng.dma_start(
                out=w_sb[:, ds(c * NN, NN)], in_=w_ap[ds(c * P, P), :]
            )
        nc.scalar.dma_start(out=b_sb[:], in_=b_ap[:])

    u_sb = small.tile([NB, D], F32)
    nc.vector.tensor_copy(u_sb[:], u_psum[:])

    # --- phase 2: transpose u/2L -> uT [D, NB] ------------------------------
    ut_psum = psum.tile([P, DCH * NB], F32)
    for c in range(DCH):
        nc.tensor.transpose(
            ut_psum[:, ds(c * NB, NB)], u_sb[:, ds(c * P, P)], ident[:]
        )
    ut_p = small.tile([P, DCH * NB], F32)
    nc.vector.tensor_copy(ut_p[:], ut_psum[:])
    ut_m = small.tile([P, DCH * NB], F32)
    nc.vector.tensor_scalar_mul(ut_m[:], ut_psum[:], -1.0)

    # --- phase 3: t_pm[n, b] = 0.5*(b[n] +- sum_d W[d,n] u[b,d]/L) ---------
    # cn-major: a PSUM bank only supports one open accumulation group.
    t_p = psum.tile([P, NCH * NB], F32)
    t_m = psum.tile([P, NCH * NB], F32)
    for tpsum, ut in ((t_p, ut_p), (t_m, ut_m)):
        for cn in range(NCH):
            for cd in range(DCH):
                nc.tensor.matmul(
                    tpsum[:, ds(cn * NB, NB)],
                    w_sb[:, ds(cd * NN + cn * P, P)],
                    ut[:, ds(cd * NB, NB)],
                    start=(cd == 0),
                    stop=False,
                )
            nc.tensor.matmul(
                tpsum[:, ds(cn * NB, NB)],
                b_sb[:, ds(cn * P, P)],
                halfones[:],
                start=False,
                stop=True,
            )

    # --- phase 4: out = relu(t_p) + relu(t_m) ------------------------------
    r_p = small.tile([P, NCH * NB], F32)
    nc.vector.tensor_scalar_max(r_p[:], t_p[:], 0.0)
    r_m = small.tile([P, NCH * NB], F32)
    nc.vector.tensor_scalar_max(r_m[:], t_m[:], 0.0)
    o_sb = small.tile([P, NCH * NB], F32)
    nc.vector.tensor_add(o_sb[:], r_p[:], r_m[:])
    nc.scalar.dma_start(out=o_view, in_=o_sb[:])


def _get_bass():
    if "nc" not in _CACHE:
        _CACHE["nc"] = _build_bass()
    return _CACHE["nc"]


def _make_in_maps(inputs):
    i = np.ascontiguousarray(np.asarray(inputs["i"], dtype=np.float32))
    j = np.ascontiguousarray(np.asarray(inputs["j"], dtype=np.float32))
    w = np.ascontiguousarray(np.asarray(inputs["W_agg"], dtype=np.float32))
    b = np.ascontiguousarray(
        np.asarray(inputs["b_agg"], dtype=np.float32).reshape(1, NN)
    )
    in_maps = []
    for c in range(NCORES):
        in_maps.append(
            {
                "i": i[c * NB : (c + 1) * NB].reshape(NB * L, D),
                "j": j[c * NB : (c + 1) * NB].reshape(NB * L, D),
                "w": w,
                "b": b,
            }
        )
    return in_maps


def run_traced(trace=False, **inputs):
    nc = _get_bass()
    in_maps = _make_in_maps(inputs)
    res = run_bass_kernel_spmd(nc, in_maps, list(range(NCORES)), trace=trace)
    out = np.concatenate(
        [res.results[c]["out"].T for c in range(NCORES)], axis=0
    ).astype(np.float32)
    return out, res


def kernel(**inputs):
    out, _ = run_traced(trace=False, **inputs)
    return out



# revision 9
# speedup vs baseline: 1.8053x; 1.8053x over previous
"""Trainium2 Bass kernel for nn_BiAlignLayer.

Reference computation:
    weight   = einsum('bld,bmd->blm', i, j)
    weight_i = softmax(weight, axis=-1)   # rows sum to 1 over m
    weight_j = softmax(weight, axis=1)    # cols sum to 1 over l
    weighted_i = einsum('blm,bld->bmd', weight_i, i)
    weighted_j = einsum('blm,bmd->bld', weight_j, j)
    oi = relu(mean_l(i - weighted_j) @ W + b)
    oj = relu(mean_m(j - weighted_i) @ W + b)
    out = 0.5 * (oi + oj)

Because mean_m(weighted_i) = mean_l(i) (softmax over m sums to 1) and
mean_l(weighted_j) = mean_m(j) (softmax over l sums to 1), the whole
attention block drops out of the final means:
    u   = mean_l(i) - mean_l(j)                       # [B, D]
    out = 0.5 * (relu(u @ W + b) + relu(-(u @ W) + b))
and with v' = (u @ W)/2 the relu pair collapses further:
    relu(x + b) + relu(b - x) == max(2b, b + |x|, 0), so
    out = max(relu(b), b/2 + |v'|)
The kernel computes exactly that shape of work; the only approximation is
that i, j and W are down-converted to float16 while being DMA'd into SBUF
(casting DMAs ride the Pool/SWDGE path). All accumulation stays in fp32
PSUM, so the end-to-end relative error is ~1e-4 against the fp32
reference -- far inside the 2e-2 gate -- while the HBM->SBUF stream
(the roofline term for this kernel) halves versus fp32.

Per-core structure:
  * 16 casting Pool DMAs stream i and j ([128, 4x512] f16 tiles, 1 MB of
    SBUF writes per batch element) back-to-back on the DMA engines.
  * The L-reduction runs "transposed" on the tensor engine: each
    [128, 128] data chunk is the matmul *stationary* and a tiny signed
    one-hot selector column (+-1/(2L), exact in f16) is the moving
    operand, accumulating uT[d, b] directly in PSUM.  Each of the 4
    d-chunk accumulation chains owns a full PSUM bank (2 KB stride) so
    the four chains can stay open concurrently across the whole stream.
    This also removes the u transpose from the tail entirely.
  * W is cast to f16 by one more Pool DMA queued after the data stream;
    b stays fp32 via a small HWDGE DMA that lands early, and the bias
    broadcasts max(b,0) / b/2 are pre-built during the stream.
  * Tail: one DVE copy-cast uT->f16, 16 tiny dense matmuls (cn-major so
    each PSUM chain closes before the next opens), and a 2-op DVE
    epilogue (|v'| + b/2, then max with relu(b)), then one output DMA.

Sharding: data-parallel over batch, 4 batch elements per core x 8 cores.
"""

import sys

import numpy as np

if "/opt/trn_rl_repo" not in sys.path:
    sys.path.insert(0, "/opt/trn_rl_repo")

import concourse.mybir as mybir
import concourse.tile as tile
from concourse import bacc
from concourse.bass import ds
from concourse.bass_utils import run_bass_kernel_spmd
from concourse.masks import make_identity

B = 32            # total batch
NCORES = 8
NB = B // NCORES  # batches per core
L = 1024
D = 512
NN = 512          # output feature dim (2 * nn_dim)
P = 128
DCH = D // P      # 128-col d-chunks
NCH = NN // P     # 128-row n-chunks
GH = 4            # 128-row groups per half-batch DMA
HLF = 2           # DMAs per (tensor, batch)
F32 = mybir.dt.float32
F16 = mybir.dt.float16
PSB = 512         # one PSUM bank, in fp32 elements per partition

_CACHE = {}


def _build_bass(reps=1):
    """Build the per-core Bass program. reps>1 repeats the body (for the
    wall-clock marginal benchmark); outputs are simply overwritten."""
    nc = bacc.Bacc("TRN2", debug=False)

    i_dram = nc.declare_dram_parameter("i", [NB * L, D], F32, isOutput=False)
    j_dram = nc.declare_dram_parameter("j", [NB * L, D], F32, isOutput=False)
    w_dram = nc.declare_dram_parameter("w", [D, NN], F32, isOutput=False)
    b_dram = nc.declare_dram_parameter("b", [1, NN], F32, isOutput=False)
    o_dram = nc.declare_dram_parameter("out", [NN, NB], F32, isOutput=True)

    # out[cn*P + p, b] <- o_sb[p, cn*NB + b]
    o_view = o_dram.ap().rearrange("(c p) b -> p c b", p=P)

    with tile.TileContext(nc) as tc:
        with (
            tc.tile_pool(name="consts", bufs=1) as consts,
            tc.tile_pool(name="data", bufs=6) as data,
            tc.tile_pool(name="small", bufs=1) as small,
            tc.tile_pool(name="psum", bufs=1, space="PSUM") as psum,
        ):
            # Signed one-hot selector columns, pre-scaled by 1/(2L) (an
            # exact power of two in f16): block b has column b = +1/(2L)
            # for i tiles, block NB+b has column b = -1/(2L) for j tiles.
            s = 1.0 / (2.0 * L)
            sel = consts.tile([P, 2 * NB * NB], F16)
            nc.vector.memset(sel[:], 0.0)
            for b in range(NB):
                nc.vector.memset(sel[:, ds(b * NB + b, 1)], s)
                nc.vector.memset(sel[:, ds((NB + b) * NB + b, 1)], -s)

            ident1 = consts.tile([1, 1], F32)
            make_identity(nc, ident1[:])
            zeros4 = consts.tile([P, NB], F32)
            nc.vector.memset(zeros4[:], 0.0)
            halfones = consts.tile([1, NB], F32)
            nc.vector.memset(halfones[:], 0.5)

            w_sb = consts.tile([P, DCH * NN], F16)
            b_sb = consts.tile([1, NN], F32)
            bb_full = consts.tile([P, NCH * NB], F32)
            bb_relu = consts.tile([P, NCH * NB], F32)

            for rep in range(reps):
                _emit_body(
                    nc, data, small, psum,
                    i_dram.ap(), j_dram.ap(), w_dram.ap(), b_dram.ap(),
                    o_view, sel, ident1, zeros4, halfones, w_sb, b_sb,
                    bb_full, bb_relu,
                    load_wb=(rep == 0),
                )

    nc.compile()
    return nc


def _emit_body(nc, data, small, psum, i_ap, j_ap, w_ap, b_ap, o_view,
               sel, ident1, zeros4, halfones, w_sb, b_sb, bb_full, bb_relu,
               load_wb=True):
    # PSUM layout (fp32 cols per partition): uT's four d-chunk accumulation
    # chains at bank stride (cols cd*512), then one bank for the dense v
    # chains and one for the bias transpose.
    ut_psum = psum.tile([P, DCH * PSB], F32)
    v_psum = psum.tile([P, PSB], F32)

    if load_wb:
        # b lands early (HWDGE on the otherwise idle SP queue) so the bias
        # broadcasts are built while the data stream runs.
        nc.sync.dma_start(out=b_sb[:], in_=b_ap[:])
        bt_psum = psum.tile([P, PSB], F32)
        for cn in range(NCH):
            nc.tensor.transpose(
                bt_psum[:, ds(cn, 1)], b_sb[:, ds(cn * P, P)], ident1[:]
            )
        for cn in range(NCH):
            nc.vector.tensor_scalar(
                bb_full[:, ds(cn * NB, NB)], zeros4[:],
                bt_psum[:, ds(cn, 1)], None, mybir.AluOpType.add,
            )
        nc.vector.tensor_scalar_max(bb_relu[:], bb_full[:], 0.0)

    # --- phase 1: uT[d, b] = (sum_l i[b,:,d] - sum_l j[b,:,d]) / 2L --------
    # Casting Pool DMAs halve the HBM->SBUF stream (the roofline term);
    # each [128, 128] f16 data chunk is then consumed as a matmul
    # *stationary* with the tiny selector column moving, so the whole
    # reduction costs the tensor engine almost nothing and produces uT in
    # the layout the dense tail wants.
    n_tiles = 2 * NB * HLF
    t_idx = 0
    for b in range(NB):
        for x_ap, blk in ((i_ap, b), (j_ap, NB + b)):
            for h in range(HLF):
                th = data.tile([P, GH * D], F16, tag="t")
                nc.gpsimd.dma_start(
                    out=th[:].rearrange("p (g n) -> p g n", g=GH),
                    in_=x_ap[ds(b * L + h * GH * P, GH * P), :].rearrange(
                        "(g p) n -> p g n", p=P
                    ),
                )
                for g in range(GH):
                    for cd in range(DCH):
                        nc.tensor.matmul(
                            ut_psum[:, ds(cd * PSB, NB)],
                            th[:, ds(g * D + cd * P, P)],
                            sel[:, ds(blk * NB, NB)],
                            start=(t_idx == 0 and g == 0),
                            stop=(t_idx == n_tiles - 1 and g == GH - 1),
                        )
                t_idx += 1

    # W is only consumed by the dense tail, so its (casting) DMA queues
    # after the data stream: the last data tile lands ~1.5us earlier and W
    # streams in while the uT copy below runs.
    if load_wb:
        nc.gpsimd.dma_start(
            out=w_sb[:].rearrange("p (c n) -> p c n", c=DCH),
            in_=w_ap.rearrange("(c p) n -> p c n", p=P),
        )

    # --- phase 2: v'[n, b] = sum_d W[d, n] uT[d, b] ------------------------
    ut_sb = small.tile([P, DCH * NB], F16)
    nc.vector.tensor_copy(
        ut_sb[:].rearrange("p (c x) -> p c x", x=NB),
        ut_psum[:].rearrange("p (c x) -> p c x", x=PSB)[:, :, ds(0, NB)],
    )
    # The bias enters PSUM as a rank-1 (K=1) matmul with a 0.5-valued rhs,
    # so v_psum holds t = v' + b/2 when the chain closes.
    for cn in range(NCH):
        for cd in range(DCH):
            nc.tensor.matmul(
                v_psum[:, ds(cn * NB, NB)],
                w_sb[:, ds(cd * NN + cn * P, P)],
                ut_sb[:, ds(cd * NB, NB)],
                start=(cd == 0),
                stop=False,
            )
        nc.tensor.matmul(
            v_psum[:, ds(cn * NB, NB)],
            b_sb[:, ds(cn * P, P)],
            halfones[:],
            start=False,
            stop=True,
        )

    # --- phase 3: out = max(relu(b), b/2 + |v'|) = max(relu(b), t, b - t) --
    tv = v_psum[:, ds(0, NCH * NB)]
    bmt = small.tile([P, NCH * NB], F32)
    nc.vector.scalar_tensor_tensor(
        bmt[:], tv, -1.0, bb_full[:], mybir.AluOpType.mult, mybir.AluOpType.add
    )
    tmp = small.tile([P, NCH * NB], F32)
    nc.vector.tensor_max(tmp[:], tv, bmt[:])
    o_sb = small.tile([P, NCH * NB], F32)
    nc.vector.tensor_max(o_sb[:], tmp[:], bb_relu[:])
    nc.sync.dma_start(out=o_view, in_=o_sb[:])


def _get_bass():
    if "nc" not in _CACHE:
        _CACHE["nc"] = _build_bass()
    return _CACHE["nc"]


def _make_in_maps(inputs):
    i = np.ascontiguousarray(np.asarray(inputs["i"], dtype=np.float32))
    j = np.ascontiguousarray(np.asarray(inputs["j"], dtype=np.float32))
    w = np.ascontiguousarray(np.asarray(inputs["W_agg"], dtype=np.float32))
    b = np.ascontiguousarray(
        np.asarray(inputs["b_agg"], dtype=np.float32).reshape(1, NN)
    )
    in_maps = []
    for c in range(NCORES):
        in_maps.append(
            {
                "i": i[c * NB : (c + 1) * NB].reshape(NB * L, D),
                "j": j[c * NB : (c + 1) * NB].reshape(NB * L, D),
                "w": w,
                "b": b,
            }
        )
    return in_maps


def run_traced(trace=False, **inputs):
    nc = _get_bass()
    in_maps = _make_in_maps(inputs)
    res = run_bass_kernel_spmd(nc, in_maps, list(range(NCORES)), trace=trace)
    out = np.concatenate(
        [res.results[c]["out"].T for c in range(NCORES)], axis=0
    ).astype(np.float32)
    return out, res


def kernel(**inputs):
    out, _ = run_traced(trace=False, **inputs)
    return out


# revision 14
# speedup vs baseline: 1.8194x; 1.0078x over previous
"""Trainium2 Bass kernel for nn_BiAlignLayer.

Reference computation:
    weight   = einsum('bld,bmd->blm', i, j)
    weight_i = softmax(weight, axis=-1)   # rows sum to 1 over m
    weight_j = softmax(weight, axis=1)    # cols sum to 1 over l
    weighted_i = einsum('blm,bld->bmd', weight_i, i)
    weighted_j = einsum('blm,bmd->bld', weight_j, j)
    oi = relu(mean_l(i - weighted_j) @ W + b)
    oj = relu(mean_m(j - weighted_i) @ W + b)
    out = 0.5 * (oi + oj)

Because mean_m(weighted_i) = mean_l(i) (softmax over m sums to 1) and
mean_l(weighted_j) = mean_m(j) (softmax over l sums to 1), the whole
attention block drops out of the final means:
    u   = mean_l(i) - mean_l(j)                       # [B, D]
    out = 0.5 * (relu(u @ W + b) + relu(-(u @ W) + b))
and with v' = (u @ W)/2 the relu pair collapses further:
    relu(x + b) + relu(b - x) == max(2b, b + |x|, 0), so
    out = max(relu(b), b/2 + |v'|)
The kernel computes exactly that shape of work; the only approximation is
that i, j and W are down-converted to float16 while being DMA'd into SBUF
(casting DMAs ride the Pool/SWDGE path). All accumulation stays in fp32
PSUM, so the end-to-end relative error is ~1e-4 against the fp32
reference -- far inside the 2e-2 gate -- while the HBM->SBUF stream
(the roofline term for this kernel) halves versus fp32.

Per-core structure:
  * 16 casting Pool DMAs stream i and j ([128, 4x512] f16 tiles, 1 MB of
    SBUF writes per batch element) back-to-back on the DMA engines.
  * The L-reduction runs "transposed" on the tensor engine: each
    [128, 128] data chunk is the matmul *stationary* and a tiny signed
    one-hot selector column (+-1/(2L), exact in f16) is the moving
    operand, accumulating uT[d, b] directly in PSUM.  Each of the 4
    d-chunk accumulation chains owns a full PSUM bank (2 KB stride) so
    the four chains can stay open concurrently across the whole stream.
    This also removes the u transpose from the tail entirely.
  * W is cast to f16 by one more Pool DMA queued after the data stream;
    b stays fp32 via a small HWDGE DMA that lands early, and the bias
    broadcasts max(b,0) / b/2 are pre-built during the stream.
  * Tail: one DVE copy-cast uT->f16, 16 tiny dense matmuls (cn-major so
    each PSUM chain closes before the next opens), and a 2-op DVE
    epilogue (|v'| + b/2, then max with relu(b)), then one output DMA.

Sharding: data-parallel over batch, 4 batch elements per core x 8 cores.
"""

import sys

import numpy as np

if "/opt/trn_rl_repo" not in sys.path:
    sys.path.insert(0, "/opt/trn_rl_repo")

import concourse.mybir as mybir
import concourse.tile as tile
from concourse import bacc
from concourse.bass import ds
from concourse.bass_utils import run_bass_kernel_spmd
from concourse.masks import make_identity

B = 32            # total batch
NCORES = 8
NB = B // NCORES  # batches per core
L = 1024
D = 512
NN = 512          # output feature dim (2 * nn_dim)
P = 128
DCH = D // P      # 128-col d-chunks
NCH = NN // P     # 128-row n-chunks
F32 = mybir.dt.float32
F16 = mybir.dt.float16
PSB = 512         # one PSUM bank, in fp32 elements per partition

_CACHE = {}


def _build_bass(reps=1):
    """Build the per-core Bass program. reps>1 repeats the body (for the
    wall-clock marginal benchmark); outputs are simply overwritten."""
    nc = bacc.Bacc("TRN2", debug=False)

    i_dram = nc.declare_dram_parameter("i", [NB * L, D], F32, isOutput=False)
    j_dram = nc.declare_dram_parameter("j", [NB * L, D], F32, isOutput=False)
    w_dram = nc.declare_dram_parameter("w", [D, NN], F32, isOutput=False)
    b_dram = nc.declare_dram_parameter("b", [1, NN], F32, isOutput=False)
    o_dram = nc.declare_dram_parameter("out", [NN, NB], F32, isOutput=True)

    # out[cn*P + p, b] <- o_sb[p, cn*NB + b]
    o_view = o_dram.ap().rearrange("(c p) b -> p c b", p=P)

    with tile.TileContext(nc) as tc:
        with (
            tc.tile_pool(name="consts", bufs=1) as consts,
            tc.tile_pool(name="data", bufs=6) as data,
            tc.tile_pool(name="small", bufs=1) as small,
            tc.tile_pool(name="psum", bufs=1, space="PSUM") as psum,
        ):
            # Signed one-hot selector columns, pre-scaled by 1/(2L) (an
            # exact power of two in f16): block b has column b = +1/(2L)
            # for i tiles, block NB+b has column b = -1/(2L) for j tiles.
            # sel32 is the f32 twin of block (i, batch 0) for the f32
            # bridge tile that rides HWDGE while the SWDGE generator spins
            # up.
            s = 1.0 / (2.0 * L)
            sel = consts.tile([P, 2 * NB * NB], F16)
            nc.vector.memset(sel[:], 0.0)
            for b in range(NB):
                nc.vector.memset(sel[:, ds(b * NB + b, 1)], s)
                nc.vector.memset(sel[:, ds((NB + b) * NB + b, 1)], -s)
            sel32 = consts.tile([P, NB], F32)
            nc.vector.memset(sel32[:], 0.0)
            nc.vector.memset(sel32[:, ds(0, 1)], s)

            ident1 = consts.tile([1, 1], F32)
            make_identity(nc, ident1[:])
            zeros4 = consts.tile([P, NB], F32)
            nc.vector.memset(zeros4[:], 0.0)
            halfones = consts.tile([1, NB], F32)
            nc.vector.memset(halfones[:], 0.5)

            w_sb = consts.tile([P, DCH * NN], F16)
            b_sb = consts.tile([1, NN], F32)
            bb_full = consts.tile([P, NCH * NB], F32)
            bb_relu = consts.tile([P, NCH * NB], F32)

            for rep in range(reps):
                _emit_body(
                    nc, data, small, psum,
                    i_dram.ap(), j_dram.ap(), w_dram.ap(), b_dram.ap(),
                    o_view, sel, sel32, ident1, zeros4, halfones, w_sb, b_sb,
                    bb_full, bb_relu,
                    load_wb=(rep == 0),
                )

    nc.compile()
    return nc


def _emit_body(nc, data, small, psum, i_ap, j_ap, w_ap, b_ap, o_view,
               sel, sel32, ident1, zeros4, halfones, w_sb, b_sb,
               bb_full, bb_relu, load_wb=True):
    # PSUM layout (fp32 cols per partition): uT's four d-chunk accumulation
    # chains at bank stride (cols cd*512), then one bank for the dense v
    # chains and one for the bias transpose.
    ut_psum = psum.tile([P, DCH * PSB], F32)
    v_psum = psum.tile([P, PSB], F32)

    # Bridge tile: the first 128 rows of batch 0's i ride a plain f32
    # HWDGE DMA.  The transfer occupies the DMA engines only during the
    # window where the first SWDGE (Pool) descriptor generation is still
    # running, so it comes to us almost free and shortens the casting
    # stream by one row-group.
    t32 = data.tile([P, D], F32, tag="t32")
    nc.sync.dma_start(out=t32[:], in_=i_ap[ds(0, P), :])

    if load_wb:
        # b lands early (HWDGE on the otherwise idle SP queue) so the bias
        # broadcasts are built while the data stream runs.
        nc.sync.dma_start(out=b_sb[:], in_=b_ap[:])
        bt_psum = psum.tile([P, PSB], F32)
        for cn in range(NCH):
            nc.tensor.transpose(
                bt_psum[:, ds(cn, 1)], b_sb[:, ds(cn * P, P)], ident1[:]
            )
        for cn in range(NCH):
            nc.vector.tensor_scalar(
                bb_full[:, ds(cn * NB, NB)], zeros4[:],
                bt_psum[:, ds(cn, 1)], None, mybir.AluOpType.add,
            )
        nc.vector.tensor_scalar_max(bb_relu[:], bb_full[:], 0.0)

    # --- phase 1: uT[d, b] = (sum_l i[b,:,d] - sum_l j[b,:,d]) / 2L --------
    # Casting Pool DMAs halve the HBM->SBUF stream (the roofline term);
    # each [128, 128] f16 data chunk is then consumed as a matmul
    # *stationary* with the tiny selector column moving, so the whole
    # reduction costs the tensor engine almost nothing and produces uT in
    # the layout the dense tail wants.  Tiles pack consecutive DRAM rows
    # per partition line, so a whole batch element is one 128-descriptor
    # DMA.  The reduction only needs every row summed once -- which rows a
    # partition holds is irrelevant -- so the row->partition mapping is
    # free to chase descriptor shape.
    #
    # The bridge covers (i, batch 0, rows 0..127): open the four cd chains
    # with its f32 matmuls.
    for cd in range(DCH):
        nc.tensor.matmul(
            ut_psum[:, ds(cd * PSB, NB)],
            t32[:, ds(cd * P, P)],
            sel32[:],
            start=True,
            stop=False,
        )
    n_tiles = 2 * NB
    t_idx = 0
    for b in range(NB):
        for x_ap, blk in ((i_ap, b), (j_ap, NB + b)):
            if t_idx == 0:
                row0, rows = b * L + P, L - P  # bridge took rows 0..127
            else:
                row0, rows = b * L, L
            rp = rows // P  # consecutive rows per partition line
            th = data.tile([P, rp * D], F16, tag="t")
            nc.gpsimd.dma_start(
                out=th[:].rearrange("p (r n) -> p r n", r=rp),
                in_=x_ap[ds(row0, rows), :].rearrange(
                    "(p r) n -> p r n", r=rp
                ),
            )
            for r in range(rp):
                for cd in range(DCH):
                    nc.tensor.matmul(
                        ut_psum[:, ds(cd * PSB, NB)],
                        th[:, ds(r * D + cd * P, P)],
                        sel[:, ds(blk * NB, NB)],
                        start=False,
                        stop=(t_idx == n_tiles - 1 and r == rp - 1),
                    )
            t_idx += 1

    # W is only consumed by the dense tail, so its (casting) DMA queues
    # after the data stream: the last data tile lands ~1.5us earlier and W
    # streams in while the uT copy below runs.
    if load_wb:
        nc.gpsimd.dma_start(
            out=w_sb[:].rearrange("p (c n) -> p c n", c=DCH),
            in_=w_ap.rearrange("(c p) n -> p c n", p=P),
        )

    # --- phase 2: v'[n, b] = sum_d W[d, n] uT[d, b] ------------------------
    ut_sb = small.tile([P, DCH * NB], F16)
    nc.vector.tensor_copy(
        ut_sb[:].rearrange("p (c x) -> p c x", x=NB),
        ut_psum[:].rearrange("p (c x) -> p c x", x=PSB)[:, :, ds(0, NB)],
    )
    # The bias enters PSUM as a rank-1 (K=1) matmul with a 0.5-valued rhs,
    # so v_psum holds t = v' + b/2 when the chain closes.
    for cn in range(NCH):
        for cd in range(DCH):
            nc.tensor.matmul(
                v_psum[:, ds(cn * NB, NB)],
                w_sb[:, ds(cd * NN + cn * P, P)],
                ut_sb[:, ds(cd * NB, NB)],
                start=(cd == 0),
                stop=False,
            )
        nc.tensor.matmul(
            v_psum[:, ds(cn * NB, NB)],
            b_sb[:, ds(cn * P, P)],
            halfones[:],
            start=False,
            stop=True,
        )

    # --- phase 3: out = max(relu(b), b/2 + |v'|) = max(relu(b), t, b - t) --
    tv = v_psum[:, ds(0, NCH * NB)]
    bmt = small.tile([P, NCH * NB], F32)
    nc.vector.scalar_tensor_tensor(
        bmt[:], tv, -1.0, bb_full[:], mybir.AluOpType.mult, mybir.AluOpType.add
    )
    tmp = small.tile([P, NCH * NB], F32)
    nc.vector.tensor_max(tmp[:], tv, bmt[:])
    o_sb = small.tile([P, NCH * NB], F32)
    nc.vector.tensor_max(o_sb[:], tmp[:], bb_relu[:])
    nc.sync.dma_start(out=o_view, in_=o_sb[:])


def _get_bass():
    if "nc" not in _CACHE:
        _CACHE["nc"] = _build_bass()
    return _CACHE["nc"]


def _make_in_maps(inputs):
    i = np.ascontiguousarray(np.asarray(inputs["i"], dtype=np.float32))
    j = np.ascontiguousarray(np.asarray(inputs["j"], dtype=np.float32))
    w = np.ascontiguousarray(np.asarray(inputs["W_agg"], dtype=np.float32))
    b = np.ascontiguousarray(
        np.asarray(inputs["b_agg"], dtype=np.float32).reshape(1, NN)
    )
    in_maps = []
    for c in range(NCORES):
        in_maps.append(
            {
                "i": i[c * NB : (c + 1) * NB].reshape(NB * L, D),
                "j": j[c * NB : (c + 1) * NB].reshape(NB * L, D),
                "w": w,
                "b": b,
            }
        )
    return in_maps


def run_traced(trace=False, **inputs):
    nc = _get_bass()
    in_maps = _make_in_maps(inputs)
    res = run_bass_kernel_spmd(nc, in_maps, list(range(NCORES)), trace=trace)
    out = np.concatenate(
        [res.results[c]["out"].T for c in range(NCORES)], axis=0
    ).astype(np.float32)
    return out, res


def kernel(**inputs):
    out, _ = run_traced(trace=False, **inputs)
    return out


# revision 19
# speedup vs baseline: 1.8285x; 1.0050x over previous
"""Trainium2 Bass kernel for nn_BiAlignLayer.

Reference computation:
    weight   = einsum('bld,bmd->blm', i, j)
    weight_i = softmax(weight, axis=-1)   # rows sum to 1 over m
    weight_j = softmax(weight, axis=1)    # cols sum to 1 over l
    weighted_i = einsum('blm,bld->bmd', weight_i, i)
    weighted_j = einsum('blm,bmd->bld', weight_j, j)
    oi = relu(mean_l(i - weighted_j) @ W + b)
    oj = relu(mean_m(j - weighted_i) @ W + b)
    out = 0.5 * (oi + oj)

Because mean_m(weighted_i) = mean_l(i) (softmax over m sums to 1) and
mean_l(weighted_j) = mean_m(j) (softmax over l sums to 1), the whole
attention block drops out of the final means:
    u   = mean_l(i) - mean_l(j)                       # [B, D]
    out = 0.5 * (relu(u @ W + b) + relu(-(u @ W) + b))
and with v' = (u @ W)/2 the relu pair collapses further:
    relu(x + b) + relu(b - x) == max(2b, b + |x|, 0), so
    out = max(relu(b), b/2 + |v'|)
The kernel computes exactly that shape of work; the only approximation is
that i, j and W are down-converted to float16 while being DMA'd into SBUF
(casting DMAs ride the Pool/SWDGE path). All accumulation stays in fp32
PSUM, so the end-to-end relative error is ~1e-4 against the fp32
reference -- far inside the 2e-2 gate -- while the HBM->SBUF stream
(the roofline term for this kernel) halves versus fp32.

Per-core structure:
  * 16 casting Pool DMAs stream i and j ([128, 4x512] f16 tiles, 1 MB of
    SBUF writes per batch element) back-to-back on the DMA engines.
  * The L-reduction runs "transposed" on the tensor engine: each
    [128, 128] data chunk is the matmul *stationary* and a tiny signed
    one-hot selector column (+-1/(2L), exact in f16) is the moving
    operand, accumulating uT[d, b] directly in PSUM.  Each of the 4
    d-chunk accumulation chains owns a full PSUM bank (2 KB stride) so
    the four chains can stay open concurrently across the whole stream.
    This also removes the u transpose from the tail entirely.
  * W is cast to f16 by one more Pool DMA queued after the data stream;
    b stays fp32 via a small HWDGE DMA that lands early, and the bias
    broadcasts max(b,0) / b/2 are pre-built during the stream.
  * Tail: one DVE copy-cast uT->f16, 16 tiny dense matmuls (cn-major so
    each PSUM chain closes before the next opens), and a 2-op DVE
    epilogue (|v'| + b/2, then max with relu(b)), then one output DMA.

Sharding: data-parallel over batch, 4 batch elements per core x 8 cores.
"""

import sys

import numpy as np

if "/opt/trn_rl_repo" not in sys.path:
    sys.path.insert(0, "/opt/trn_rl_repo")

import concourse.mybir as mybir
import concourse.tile as tile
from concourse import bacc
from concourse.bass import ds
from concourse.bass_utils import run_bass_kernel_spmd

B = 32            # total batch
NCORES = 8
NB = B // NCORES  # batches per core
L = 1024
D = 512
NN = 512          # output feature dim (2 * nn_dim)
P = 128
DCH = D // P      # 128-col d-chunks
NCH = NN // P     # 128-row n-chunks
F32 = mybir.dt.float32
F16 = mybir.dt.float16
PSB = 512         # one PSUM bank, in fp32 elements per partition

_CACHE = {}


def _build_bass(reps=1):
    """Build the per-core Bass program. reps>1 repeats the body (for the
    wall-clock marginal benchmark); outputs are simply overwritten."""
    nc = bacc.Bacc("TRN2", debug=False)

    i_dram = nc.declare_dram_parameter("i", [NB * L, D], F32, isOutput=False)
    j_dram = nc.declare_dram_parameter("j", [NB * L, D], F32, isOutput=False)
    w_dram = nc.declare_dram_parameter("w", [D, NN], F32, isOutput=False)
    b_dram = nc.declare_dram_parameter("b", [1, NN], F32, isOutput=False)
    o_dram = nc.declare_dram_parameter("out", [NN, NB], F32, isOutput=True)

    # out[cn*P + p, b] <- o_sb[p, cn*NB + b]
    o_view = o_dram.ap().rearrange("(c p) b -> p c b", p=P)

    with tile.TileContext(nc) as tc:
        with (
            tc.tile_pool(name="consts", bufs=1) as consts,
            tc.tile_pool(name="data", bufs=6) as data,
            tc.tile_pool(name="small", bufs=1) as small,
            tc.tile_pool(name="psum", bufs=1, space="PSUM") as psum,
        ):
            # Signed one-hot selector columns, pre-scaled by 1/(2L) (an
            # exact power of two in f16): block b has column b = +1/(2L)
            # for i tiles, block NB+b has column b = -1/(2L) for j tiles.
            # sel32 is the f32 twin of block (i, batch 0) for the f32
            # bridge tile that rides HWDGE while the SWDGE generator spins
            # up.
            s = 1.0 / (2.0 * L)
            sel = consts.tile([P, 2 * NB * NB], F16)
            nc.vector.memset(sel[:], 0.0)
            for b in range(NB):
                nc.vector.memset(sel[:, ds(b * NB + b, 1)], s)
                nc.vector.memset(sel[:, ds((NB + b) * NB + b, 1)], -s)
            sel32 = consts.tile([P, NB], F32)
            nc.vector.memset(sel32[:], 0.0)
            nc.vector.memset(sel32[:, ds(0, 1)], s)

            halfones = consts.tile([1, NB], F32)
            nc.vector.memset(halfones[:], 0.5)

            w_sb = consts.tile([P, DCH * NN], F16)
            b_sb = consts.tile([1, NN], F32)

            for rep in range(reps):
                _emit_body(
                    nc, data, small, psum,
                    i_dram.ap(), j_dram.ap(), w_dram.ap(), b_dram.ap(),
                    o_view, sel, sel32, halfones, w_sb, b_sb,
                    load_wb=(rep == 0),
                )

    nc.compile()
    return nc


def _emit_body(nc, data, small, psum, i_ap, j_ap, w_ap, b_ap, o_view,
               sel, sel32, halfones, w_sb, b_sb, load_wb=True):
    # PSUM layout (fp32 cols per partition): uT's four d-chunk accumulation
    # chains at bank stride (cols cd*512), then one bank each for the
    # dense t = (v+b)/2 and m = (b-v)/2 chains.
    ut_psum = psum.tile([P, DCH * PSB], F32)
    t_psum = psum.tile([P, PSB], F32)
    m_psum = psum.tile([P, PSB], F32)

    # Bridge tile: the first 128 rows of batch 0's i ride a plain f32
    # HWDGE DMA.  The transfer occupies the DMA engines only during the
    # window where the first SWDGE (Pool) descriptor generation is still
    # running, so it comes to us almost free and shortens the casting
    # stream by one row-group.
    t32 = data.tile([P, D], F32, tag="t32")
    nc.sync.dma_start(out=t32[:], in_=i_ap[ds(0, P), :])

    if load_wb:
        # b is tiny and only feeds the rank-1 bias matmuls of the tail.
        nc.sync.dma_start(out=b_sb[:], in_=b_ap[:])

    # --- phase 1: uT[d, b] = (sum_l i[b,:,d] - sum_l j[b,:,d]) / 2L --------
    # Casting Pool DMAs halve the HBM->SBUF stream (the roofline term);
    # each [128, 128] f16 data chunk is then consumed as a matmul
    # *stationary* with the tiny selector column moving, so the whole
    # reduction costs the tensor engine almost nothing and produces uT in
    # the layout the dense tail wants.  Tiles pack consecutive DRAM rows
    # per partition line, so a whole batch element is one 128-descriptor
    # DMA.  The reduction only needs every row summed once -- which rows a
    # partition holds is irrelevant -- so the row->partition mapping is
    # free to chase descriptor shape.
    #
    # The bridge covers (i, batch 0, rows 0..127): open the four cd chains
    # with its f32 matmuls.
    for cd in range(DCH):
        nc.tensor.matmul(
            ut_psum[:, ds(cd * PSB, NB)],
            t32[:, ds(cd * P, P)],
            sel32[:],
            start=True,
            stop=False,
        )
    n_tiles = 2 * NB
    t_idx = 0
    for b in range(NB):
        for x_ap, blk in ((i_ap, b), (j_ap, NB + b)):
            if t_idx == 0:
                row0, rows = b * L + P, L - P  # bridge took rows 0..127
            else:
                row0, rows = b * L, L
            rp = rows // P  # consecutive rows per partition line
            th = data.tile([P, rp * D], F16, tag="t")
            nc.gpsimd.dma_start(
                out=th[:].rearrange("p (r n) -> p r n", r=rp),
                in_=x_ap[ds(row0, rows), :].rearrange(
                    "(p r) n -> p r n", r=rp
                ),
            )
            for r in range(rp):
                for cd in range(DCH):
                    nc.tensor.matmul(
                        ut_psum[:, ds(cd * PSB, NB)],
                        th[:, ds(r * D + cd * P, P)],
                        sel[:, ds(blk * NB, NB)],
                        start=False,
                        stop=(t_idx == n_tiles - 1 and r == rp - 1),
                    )
            t_idx += 1

    # W is only consumed by the dense tail, so its (casting) DMA queues
    # after the data stream: the last data tile lands ~1.5us earlier and W
    # streams in while the uT copy below runs.
    if load_wb:
        nc.gpsimd.dma_start(
            out=w_sb[:].rearrange("p (c n) -> p c n", c=DCH),
            in_=w_ap.rearrange("(c p) n -> p c n", p=P),
        )

    # --- phase 2: t[n,b] = (v+b)/2, m[n,b] = (b-v)/2, v = sum_d W[d,n] u[b,d]
    # The +-uT copies run right after the last data tile and hide inside
    # the W DMA + sem window; the bias enters each PSUM chain as a rank-1
    # (K=1) matmul with a 0.5-valued rhs.
    ut_sb = small.tile([P, DCH * NB], F16)
    nc.vector.tensor_copy(
        ut_sb[:].rearrange("p (c x) -> p c x", x=NB),
        ut_psum[:].rearrange("p (c x) -> p c x", x=PSB)[:, :, ds(0, NB)],
    )
    utn_sb = small.tile([P, DCH * NB], F16)
    nc.vector.tensor_scalar_mul(
        utn_sb[:].rearrange("p (c x) -> p c x", x=NB),
        ut_psum[:].rearrange("p (c x) -> p c x", x=PSB)[:, :, ds(0, NB)],
        -1.0,
    )
    for tpsum, ut in ((t_psum, ut_sb), (m_psum, utn_sb)):
        for cn in range(NCH):
            for cd in range(DCH):
                nc.tensor.matmul(
                    tpsum[:, ds(cn * NB, NB)],
                    w_sb[:, ds(cd * NN + cn * P, P)],
                    ut[:, ds(cd * NB, NB)],
                    start=(cd == 0),
                    stop=False,
                )
            nc.tensor.matmul(
                tpsum[:, ds(cn * NB, NB)],
                b_sb[:, ds(cn * P, P)],
                halfones[:],
                start=False,
                stop=True,
            )

    # --- phase 3: out = relu(t) + relu(m) ----------------------------------
    # The two relu-maxes are independent (separate PSUM banks), so they
    # pipeline on the DVE with no side-effect stall between them.
    r_t = small.tile([P, NCH * NB], F32)
    nc.vector.tensor_scalar_max(r_t[:], t_psum[:, ds(0, NCH * NB)], 0.0)
    r_m = small.tile([P, NCH * NB], F32)
    nc.vector.tensor_scalar_max(r_m[:], m_psum[:, ds(0, NCH * NB)], 0.0)
    o_sb = small.tile([P, NCH * NB], F32)
    nc.vector.tensor_add(o_sb[:], r_t[:], r_m[:])
    nc.sync.dma_start(out=o_view, in_=o_sb[:])


def _get_bass():
    if "nc" not in _CACHE:
        _CACHE["nc"] = _build_bass()
    return _CACHE["nc"]


def _make_in_maps(inputs):
    i = np.ascontiguousarray(np.asarray(inputs["i"], dtype=np.float32))
    j = np.ascontiguousarray(np.asarray(inputs["j"], dtype=np.float32))
    w = np.ascontiguousarray(np.asarray(inputs["W_agg"], dtype=np.float32))
    b = np.ascontiguousarray(
        np.asarray(inputs["b_agg"], dtype=np.float32).reshape(1, NN)
    )
    in_maps = []
    for c in range(NCORES):
        in_maps.append(
            {
                "i": i[c * NB : (c + 1) * NB].reshape(NB * L, D),
                "j": j[c * NB : (c + 1) * NB].reshape(NB * L, D),
                "w": w,
                "b": b,
            }
        )
    return in_maps


def run_traced(trace=False, **inputs):
    nc = _get_bass()
    in_maps = _make_in_maps(inputs)
    res = run_bass_kernel_spmd(nc, in_maps, list(range(NCORES)), trace=trace)
    out = np.concatenate(
        [res.results[c]["out"].T for c in range(NCORES)], axis=0
    ).astype(np.float32)
    return out, res


def kernel(**inputs):
    out, _ = run_traced(trace=False, **inputs)
    return out


# revision 22
# speedup vs baseline: 1.8343x; 1.0032x over previous
"""Trainium2 Bass kernel for nn_BiAlignLayer.

Reference computation:
    weight   = einsum('bld,bmd->blm', i, j)
    weight_i = softmax(weight, axis=-1)   # rows sum to 1 over m
    weight_j = softmax(weight, axis=1)    # cols sum to 1 over l
    weighted_i = einsum('blm,bld->bmd', weight_i, i)
    weighted_j = einsum('blm,bmd->bld', weight_j, j)
    oi = relu(mean_l(i - weighted_j) @ W + b)
    oj = relu(mean_m(j - weighted_i) @ W + b)
    out = 0.5 * (oi + oj)

Because mean_m(weighted_i) = mean_l(i) (softmax over m sums to 1) and
mean_l(weighted_j) = mean_m(j) (softmax over l sums to 1), the whole
attention block drops out of the final means:
    u   = mean_l(i) - mean_l(j)                       # [B, D]
    out = 0.5 * (relu(u @ W + b) + relu(-(u @ W) + b))
and with v' = (u @ W)/2 the relu pair collapses further:
    relu(x + b) + relu(b - x) == max(2b, b + |x|, 0), so
    out = max(relu(b), b/2 + |v'|)
The kernel computes exactly that shape of work; the only approximation is
that i, j and W are down-converted to float16 while being DMA'd into SBUF
(casting DMAs ride the Pool/SWDGE path). All accumulation stays in fp32
PSUM, so the end-to-end relative error is ~1e-4 against the fp32
reference -- far inside the 2e-2 gate -- while the HBM->SBUF stream
(the roofline term for this kernel) halves versus fp32.

Per-core structure:
  * 16 casting Pool DMAs stream i and j ([128, 4x512] f16 tiles, 1 MB of
    SBUF writes per batch element) back-to-back on the DMA engines.
  * The L-reduction runs "transposed" on the tensor engine: each
    [128, 128] data chunk is the matmul *stationary* and a tiny signed
    one-hot selector column (+-1/(2L), exact in f16) is the moving
    operand, accumulating uT[d, b] directly in PSUM.  Each of the 4
    d-chunk accumulation chains owns a full PSUM bank (2 KB stride) so
    the four chains can stay open concurrently across the whole stream.
    This also removes the u transpose from the tail entirely.
  * W is cast to f16 by one more Pool DMA queued after the data stream;
    b stays fp32 via a small HWDGE DMA that lands early, and the bias
    broadcasts max(b,0) / b/2 are pre-built during the stream.
  * Tail: one DVE copy-cast uT->f16, 16 tiny dense matmuls (cn-major so
    each PSUM chain closes before the next opens), and a 2-op DVE
    epilogue (|v'| + b/2, then max with relu(b)), then one output DMA.

Sharding: data-parallel over batch, 4 batch elements per core x 8 cores.
"""

import sys

import numpy as np

if "/opt/trn_rl_repo" not in sys.path:
    sys.path.insert(0, "/opt/trn_rl_repo")

import concourse.mybir as mybir
import concourse.tile as tile
from concourse import bacc
from concourse.bass import ds
from concourse.bass_utils import run_bass_kernel_spmd

B = 32            # total batch
NCORES = 8
NB = B // NCORES  # batches per core
L = 1024
D = 512
NN = 512          # output feature dim (2 * nn_dim)
P = 128
DCH = D // P      # 128-col d-chunks
NCH = NN // P     # 128-row n-chunks
F32 = mybir.dt.float32
F16 = mybir.dt.float16
PSB = 512         # one PSUM bank, in fp32 elements per partition

_CACHE = {}


def _build_bass(reps=1):
    """Build the per-core Bass program. reps>1 repeats the body (for the
    wall-clock marginal benchmark); outputs are simply overwritten."""
    nc = bacc.Bacc("TRN2", debug=False)

    i_dram = nc.declare_dram_parameter("i", [NB * L, D], F32, isOutput=False)
    j_dram = nc.declare_dram_parameter("j", [NB * L, D], F32, isOutput=False)
    w_dram = nc.declare_dram_parameter("w", [D, NN], F32, isOutput=False)
    b_dram = nc.declare_dram_parameter("b", [1, NN], F32, isOutput=False)
    o_dram = nc.declare_dram_parameter("out", [NN, NB], F32, isOutput=True)

    # out[cn*P + p, b] <- o_sb[p, cn*NB + b]
    o_view = o_dram.ap().rearrange("(c p) b -> p c b", p=P)

    with tile.TileContext(nc) as tc:
        with (
            tc.tile_pool(name="consts", bufs=1) as consts,
            tc.tile_pool(name="data", bufs=6) as data,
            tc.tile_pool(name="small", bufs=1) as small,
            tc.tile_pool(name="psum", bufs=1, space="PSUM") as psum,
        ):
            # Signed one-hot selector columns, pre-scaled by 1/(2L) (an
            # exact power of two in f16): block b has column b = +1/(2L)
            # for i tiles, block NB+b has column b = -1/(2L) for j tiles.
            # sel32 is the f32 twin of block (i, batch 0) for the f32
            # bridge tile that rides HWDGE while the SWDGE generator spins
            # up.
            s = 1.0 / (2.0 * L)
            sel = consts.tile([P, 2 * NB * NB], F16)
            nc.vector.memset(sel[:], 0.0)
            for b in range(NB):
                nc.vector.memset(sel[:, ds(b * NB + b, 1)], s)
                nc.vector.memset(sel[:, ds((NB + b) * NB + b, 1)], -s)
            sel32 = consts.tile([P, NB], F32)
            nc.vector.memset(sel32[:], 0.0)
            nc.vector.memset(sel32[:, ds(0, 1)], s)

            halfones = consts.tile([1, 2 * NB], F32)
            nc.vector.memset(halfones[:], 0.5)

            w_sb = consts.tile([P, DCH * NN], F16)
            b_sb = consts.tile([1, NN], F32)

            for rep in range(reps):
                _emit_body(
                    nc, data, small, psum,
                    i_dram.ap(), j_dram.ap(), w_dram.ap(), b_dram.ap(),
                    o_view, sel, sel32, halfones, w_sb, b_sb,
                    load_wb=(rep == 0),
                )

    nc.compile()
    return nc


def _emit_body(nc, data, small, psum, i_ap, j_ap, w_ap, b_ap, o_view,
               sel, sel32, halfones, w_sb, b_sb, load_wb=True):
    # PSUM layout (fp32 cols per partition): uT's four d-chunk accumulation
    # chains at bank stride (cols cd*512), then one bank holding the dense
    # t = (v+b)/2 and m = (b-v)/2 results interleaved per n-chunk.
    ut_psum = psum.tile([P, DCH * PSB], F32)
    tm_psum = psum.tile([P, PSB], F32)

    # Bridge tile: the first 128 rows of batch 0's i ride a plain f32
    # HWDGE DMA.  The transfer occupies the DMA engines only during the
    # window where the first SWDGE (Pool) descriptor generation is still
    # running, so it comes to us almost free and shortens the casting
    # stream by one row-group.
    t32 = data.tile([P, D], F32, tag="t32")
    nc.sync.dma_start(out=t32[:], in_=i_ap[ds(0, P), :])

    if load_wb:
        # b is tiny and only feeds the rank-1 bias matmuls of the tail.
        nc.sync.dma_start(out=b_sb[:], in_=b_ap[:])

    # --- phase 1: uT[d, b] = (sum_l i[b,:,d] - sum_l j[b,:,d]) / 2L --------
    # Casting Pool DMAs halve the HBM->SBUF stream (the roofline term);
    # each [128, 128] f16 data chunk is then consumed as a matmul
    # *stationary* with the tiny selector column moving, so the whole
    # reduction costs the tensor engine almost nothing and produces uT in
    # the layout the dense tail wants.  Tiles pack consecutive DRAM rows
    # per partition line, so a whole batch element is one 128-descriptor
    # DMA.  The reduction only needs every row summed once -- which rows a
    # partition holds is irrelevant -- so the row->partition mapping is
    # free to chase descriptor shape.
    #
    # The bridge covers (i, batch 0, rows 0..127): open the four cd chains
    # with its f32 matmuls.
    for cd in range(DCH):
        nc.tensor.matmul(
            ut_psum[:, ds(cd * PSB, NB)],
            t32[:, ds(cd * P, P)],
            sel32[:],
            start=True,
            stop=False,
        )
    n_tiles = 2 * NB
    t_idx = 0
    for b in range(NB):
        for x_ap, blk in ((i_ap, b), (j_ap, NB + b)):
            if t_idx == 0:
                row0, rows = b * L + P, L - P  # bridge took rows 0..127
            else:
                row0, rows = b * L, L
            rp = rows // P  # consecutive rows per partition line
            th = data.tile([P, rp * D], F16, tag="t")
            nc.gpsimd.dma_start(
                out=th[:].rearrange("p (r n) -> p r n", r=rp),
                in_=x_ap[ds(row0, rows), :].rearrange(
                    "(p r) n -> p r n", r=rp
                ),
            )
            for r in range(rp):
                for cd in range(DCH):
                    nc.tensor.matmul(
                        ut_psum[:, ds(cd * PSB, NB)],
                        th[:, ds(r * D + cd * P, P)],
                        sel[:, ds(blk * NB, NB)],
                        start=False,
                        stop=(t_idx == n_tiles - 1 and r == rp - 1),
                    )
            t_idx += 1

    # W is only consumed by the dense tail, so its (casting) DMA queues
    # after the data stream: the last data tile lands ~1.5us earlier and W
    # streams in while the uT copy below runs.
    if load_wb:
        nc.gpsimd.dma_start(
            out=w_sb[:].rearrange("p (c n) -> p c n", c=DCH),
            in_=w_ap.rearrange("(c p) n -> p c n", p=P),
        )

    # --- phase 2: t[n,b] = (v+b)/2, m[n,b] = (b-v)/2, v = sum_d W[d,n] u[b,d]
    # +uT and -uT sit side by side in one [128, 8]-moving tile, so t and m
    # come from a single matmul chain per n-chunk (the stationary W block
    # is shared) and land interleaved in one PSUM bank.  The copies run
    # right after the last data tile and hide inside the W DMA + sem
    # window; the bias enters each chain as a rank-1 (K=1) matmul with a
    # 0.5-valued rhs.
    ut_view = ut_psum[:].rearrange("p (c x) -> p c x", x=PSB)[:, :, ds(0, NB)]
    ut_pm = small.tile([P, DCH * 2 * NB], F16)
    pm_view = ut_pm[:].rearrange("p (c s x) -> p c s x", s=2, x=NB)
    nc.vector.tensor_copy(pm_view[:, :, 0, :], ut_view)
    nc.vector.tensor_scalar_mul(pm_view[:, :, 1, :], ut_view, -1.0)
    for cn in range(NCH):
        for cd in range(DCH):
            nc.tensor.matmul(
                tm_psum[:, ds(cn * 2 * NB, 2 * NB)],
                w_sb[:, ds(cd * NN + cn * P, P)],
                ut_pm[:, ds(cd * 2 * NB, 2 * NB)],
                start=(cd == 0),
                stop=False,
            )
        nc.tensor.matmul(
            tm_psum[:, ds(cn * 2 * NB, 2 * NB)],
            b_sb[:, ds(cn * P, P)],
            halfones[:],
            start=False,
            stop=True,
        )

    # --- phase 3: out = relu(t) + relu(m) ----------------------------------
    r_tm = small.tile([P, NCH * 2 * NB], F32)
    nc.vector.tensor_scalar_max(r_tm[:], tm_psum[:, ds(0, NCH * 2 * NB)], 0.0)
    r_view = r_tm[:].rearrange("p (c s x) -> p c s x", s=2, x=NB)
    o_sb = small.tile([P, NCH * NB], F32)
    nc.vector.tensor_add(
        o_sb[:].rearrange("p (c x) -> p c x", x=NB),
        r_view[:, :, 0, :],
        r_view[:, :, 1, :],
    )
    nc.sync.dma_start(out=o_view, in_=o_sb[:])


def _get_bass():
    if "nc" not in _CACHE:
        _CACHE["nc"] = _build_bass()
    return _CACHE["nc"]


def _make_in_maps(inputs):
    i = np.ascontiguousarray(np.asarray(inputs["i"], dtype=np.float32))
    j = np.ascontiguousarray(np.asarray(inputs["j"], dtype=np.float32))
    w = np.ascontiguousarray(np.asarray(inputs["W_agg"], dtype=np.float32))
    b = np.ascontiguousarray(
        np.asarray(inputs["b_agg"], dtype=np.float32).reshape(1, NN)
    )
    in_maps = []
    for c in range(NCORES):
        in_maps.append(
            {
                "i": i[c * NB : (c + 1) * NB].reshape(NB * L, D),
                "j": j[c * NB : (c + 1) * NB].reshape(NB * L, D),
                "w": w,
                "b": b,
            }
        )
    return in_maps


def run_traced(trace=False, **inputs):
    nc = _get_bass()
    in_maps = _make_in_maps(inputs)
    res = run_bass_kernel_spmd(nc, in_maps, list(range(NCORES)), trace=trace)
    out = np.concatenate(
        [res.results[c]["out"].T for c in range(NCORES)], axis=0
    ).astype(np.float32)
    return out, res


def kernel(**inputs):
    out, _ = run_traced(trace=False, **inputs)
    return out
